# revision 3
# baseline (speedup 1.0000x reference)
"""Trainium2 Bass kernel: Atom2Residue (gnn_message_passing).

Math: out[n,c,o] = sum_i fuse[n,c,i] * w[l(c),o,i]  (+ b[o] at c==0)
where fuse[n,c,:] = concat(CA_atom[n,c,:16], res_emb[n,c,:32]), l(c)=floor(sqrt(c)).

Strategy (8 cores, data parallel over 18750 residues/core, no collectives):
  - Host gathers the CA atoms, fuses with res_emb, and pre-packs the per-core
    input TRANSPOSED (channels-on-partitions) in bf16: fin[432, 18944] as 4
    coefficient-pair groups of 96 rows + the c8 group of 48 rows. The device
    does zero on-chip transposes.
  - Per 2048-residue super-chunk: 2 input DMAs, then per 512-residue chunk 5
    block-diagonal matmuls (2 coefs each, K=96/48, bf16, start&stop, no PSUM
    accumulation), PSUM f32 -> SBUF bf16 cast copies split across scalar and
    vector engines, 2 output DMAs per super. Outputs stored transposed; host
    unscrambles, upcasts to f32, adds the l=0 bias.
  - ~41 DMAs/core total (HWDGE descriptor issue is ~625 ns each, serialized
    per ring); input DMAs ride the SP HWDGE ring, output stores the ACT ring
    so stores never head-of-line-block input prefetch.
  - The c8 input group sits on SBUF partitions 64:112 (not 0:48) to balance
    the per-AXI-port byte load: ports are hardwired to partition groups, and
    the main input only covers partitions 0:95.
  - Device HBM traffic: 27.3 MB/core bf16 -> 76 us roofline at 358 GB/s;
    measured ~73-77 us steady-state (vs 1262 us f32 natural-layout baseline).
"""

import os
import sys

for _p in ("/opt/trn_rl_repo",):
    if os.path.isdir(_p) and _p not in sys.path:
        sys.path.insert(0, _p)

import numpy as np

from concourse import bacc, bass, mybir  # noqa: F401
from concourse.bass_utils import run_bass_kernel_spmd
from concourse.tile import TileContext

F32 = mybir.dt.float32
BF16 = mybir.dt.bfloat16
NP_BF16 = mybir.dt.np(mybir.dt.bfloat16)

NUM_COEF, ATOM_C, NODE_C = 9, 16, 32
COEF_C = ATOM_C + NODE_C            # 48 fused in-channels per coefficient
L_OF_COEF = np.floor(np.sqrt(np.arange(NUM_COEF))).astype(np.int64)

N_CORES = 8
R_TOTAL = 150_000
RS = R_TOTAL // N_CORES             # 18750 residues per core
CH = 512                            # residues per chunk (PSUM bank width f32)
NCH = -(-RS // CH)                  # 37 chunks
NPAD = NCH * CH                     # 18944 padded residues per core
FIN_ROWS = NUM_COEF * COEF_C        # 432 input rows (4 groups of 96 + 48)
NSLAB = -(-NCH // 4)                # 10 c8-output slabs of 4 chunks each
WSB_COLS = 4 * 64 + 32              # 288 stationary-weight columns


def build_wsb(w):
    """[128, 288] bf16: 4 groups of block-diag(w[l(2g)].T, w[l(2g+1)].T) 64
    cols each at partition base 0, then w[2].T for coef 8 at col 256 on
    partitions 64:112 (c8 rides the upper partitions to balance the SBUF AXI
    port load of its DMA against the 96-partition main input)."""
    w = np.asarray(w, np.float32)
    wsb = np.zeros((128, WSB_COLS), np.float32)
    for g in range(4):
        for half in range(2):
            c = 2 * g + half
            blk = w[L_OF_COEF[c]].T        # [48 in, 32 out]
            wsb[48 * half:48 * half + 48, 64 * g + 32 * half:64 * g + 32 * half + 32] = blk
    wsb[64:112, 256:288] = w[2].T
    return wsb.astype(NP_BF16)


SUP = 4 * CH                 # 2048-residue super-chunk = one c8 output slab


def build_nc(nch=NCH, reps=1):
    nc = bacc.Bacc()
    npad = nch * CH
    nsup = -(-nch // 4)
    # super widths: 4 chunks each, last one may be partial
    sups = []
    left = nch
    while left > 0:
        w = min(4, left)
        sups.append(w * CH)
        left -= w
    fin_d = nc.declare_dram_parameter("fin", [FIN_ROWS, npad], BF16, isOutput=False)
    wsb_d = nc.declare_dram_parameter("wsb", [128, WSB_COLS], BF16, isOutput=False)
    outAB_d = nc.declare_dram_parameter("outAB", [128, 2 * npad], BF16,
                                        isOutput=True)
    outC_d = nc.declare_dram_parameter("outC", [128, nsup * CH], BF16,
                                       isOutput=True)

    with TileContext(nc) as tc:
        with (
            tc.tile_pool(name="const", bufs=1) as cpool,
            tc.tile_pool(name="fin", bufs=3) as fin_pool,
            tc.tile_pool(name="f4", bufs=3) as f4_pool,
            tc.tile_pool(name="oAB", bufs=3) as oAB_pool,
            tc.tile_pool(name="oC", bufs=2) as oC_pool,
            tc.tile_pool(name="pA", bufs=2, space="PSUM") as pA_pool,
            tc.tile_pool(name="pB", bufs=2, space="PSUM") as pB_pool,
            tc.tile_pool(name="pC", bufs=2, space="PSUM") as pC_pool,
        ):
            wsb_sb = cpool.tile([128, WSB_COLS], BF16)
            nc.sync.dma_start(out=wsb_sb[:], in_=wsb_d[:])

            for s, w in [(s, w) for _ in range(reps) for s, w in enumerate(sups)]:
                c0 = SUP * s                 # residue-column base of the super
                nj = w // CH                 # chunks in this super (4 or less)
                big = fin_pool.tile([96, 4 * w], BF16, tag="big")
                nc.sync.dma_start(
                    out=big[:].rearrange("p (g n) -> p g n", g=4),
                    in_=fin_d[0:384, c0:c0 + w].rearrange("(g p) n -> p g n", g=4),
                )
                f4 = f4_pool.tile([112, w], BF16, tag="f4")
                nc.sync.dma_start(out=f4[64:112, :], in_=fin_d[384:432, c0:c0 + w])

                oAB = oAB_pool.tile([128, 2 * w], BF16, tag="oAB")
                pC = pC_pool.tile([128, CH], F32, tag="pC")
                for j in range(nj):
                    pA = pA_pool.tile([128, CH], F32, tag="pA")
                    pB = pB_pool.tile([128, CH], F32, tag="pB")
                    for g in range(4):
                        ob = 64 * (g % 2)
                        psum = pA if g < 2 else pB
                        nc.tensor.matmul(
                            psum[ob:ob + 64, :],
                            wsb_sb[0:96, 64 * g:64 * g + 64],
                            big[0:96, w * g + CH * j:w * g + CH * (j + 1)],
                            start=True, stop=True,
                            skip_group_check=True,
                            tile_position=(0, ob),
                        )
                    nc.tensor.matmul(
                        pC[32 * j:32 * j + 32, :],
                        wsb_sb[64:112, 256:288],
                        f4[64:112, CH * j:CH * (j + 1)],
                        start=True, stop=True,
                        skip_group_check=True,
                        tile_position=(64, 32 * j),
                    )
                    nc.scalar.copy(out=oAB[:, 2 * CH * j:2 * CH * j + CH],
                                   in_=pA[:])
                    nc.vector.tensor_copy(
                        oAB[:, 2 * CH * j + CH:2 * CH * (j + 1)], pB[:])

                nc.scalar.dma_start(out=outAB_d[:, 2 * c0:2 * (c0 + w)],
                                    in_=oAB[:])
                rows = 32 * nj
                oC = oC_pool.tile([128, CH], BF16, tag="oC")
                if s % 2 == 0:
                    nc.scalar.copy(out=oC[0:rows, :], in_=pC[0:rows, :])
                else:
                    nc.vector.tensor_copy(oC[0:rows, :], pC[0:rows, :])
                nc.scalar.dma_start(out=outC_d[0:rows, CH * s:CH * (s + 1)],
                                    in_=oC[0:rows, :])
    nc.finalize()
    return nc


_NC_CACHE = {}


def _get_nc(nch=NCH):
    if nch not in _NC_CACHE:
        _NC_CACHE[nch] = build_nc(nch)
    return _NC_CACHE[nch]


def _make_in_maps(atom_agg, res_emb, w, b, backbone_idx, ca_res_idx):
    atom_agg = np.ascontiguousarray(np.asarray(atom_agg, dtype=np.float32))
    res_emb = np.ascontiguousarray(np.asarray(res_emb, dtype=np.float32))
    backbone_idx = np.asarray(backbone_idx)
    ca_res_idx = np.asarray(ca_res_idx)
    num_res = res_emb.shape[0]
    assert num_res == R_TOTAL, f"kernel compiled for {R_TOTAL} residues"

    wsb = build_wsb(w)
    A2 = atom_agg.reshape(atom_agg.shape[0], NUM_COEF * ATOM_C)
    ca_atom = backbone_idx.reshape(-1, 4)[:, 1]
    cont = np.zeros((num_res, NUM_COEF * ATOM_C), np.float32)
    cont[ca_res_idx] = A2[ca_atom]

    # bf16 cast in natural layout (contiguous), then one transpose copy into
    # channels-on-partitions [core, 9, 48, NPAD] -> [core, 432, NPAD]
    tmp = np.zeros((N_CORES, NPAD, NUM_COEF, COEF_C), NP_BF16)
    tmp[:, :RS, :, 0:ATOM_C] = cont.reshape(N_CORES, RS, NUM_COEF, ATOM_C)
    tmp[:, :RS, :, ATOM_C:] = res_emb.reshape(N_CORES, RS, NUM_COEF, NODE_C)
    fin_all = np.ascontiguousarray(tmp.transpose(0, 2, 3, 1)).reshape(
        N_CORES, FIN_ROWS, NPAD)

    return [{"fin": fin_all[c], "wsb": wsb} for c in range(N_CORES)]


def _gather_out(results, b):
    b = np.asarray(b, np.float32)
    out = np.empty((N_CORES, RS, NUM_COEF, NODE_C), np.float32)
    for c in range(N_CORES):
        r = results[c]
        # outAB: row 32cc+o, col 1024t + 512h + n ; coef = 4h + cc
        ab = r["outAB"].reshape(4, NODE_C, NCH, 2, CH).transpose(2, 4, 3, 0, 1)
        ab = ab.reshape(NPAD, 2, 4, NODE_C)[:RS]      # [n, h, cc, o]
        out[c, :, 0:4] = np.asarray(ab[:, 0], np.float32)
        out[c, :, 4:8] = np.asarray(ab[:, 1], np.float32)
        # outC: [4(j), 32, NSLAB, 512] ; chunk k=4s+j, n=512k+i
        cC = r["outC"].reshape(4, NODE_C, NSLAB, CH).transpose(2, 0, 3, 1)
        out[c, :, 8] = np.asarray(cC.reshape(NSLAB * 4 * CH, NODE_C)[:RS],
                                  np.float32)
    out = out.reshape(R_TOTAL, NUM_COEF, NODE_C)
    out[:, 0, :] += b
    return out


def _run(in_maps, trace=False, **kw):
    nc = _get_nc()
    return run_bass_kernel_spmd(nc, in_maps, core_ids=list(range(N_CORES)),
                                trace=trace, **kw)


def kernel(atom_agg, res_emb, w, b, backbone_idx, ca_res_idx):
    in_maps = _make_in_maps(atom_agg, res_emb, w, b, backbone_idx, ca_res_idx)
    res = _run(in_maps, trace=False)
    return _gather_out(res.results, b)


def kernel_profiled(atom_agg, res_emb, w, b, backbone_idx, ca_res_idx, **kw):
    """Same as kernel() but requests an NTFF trace; returns (out, BassKernelResults)."""
    in_maps = _make_in_maps(atom_agg, res_emb, w, b, backbone_idx, ca_res_idx)
    res = _run(in_maps, trace=True, **kw)
    return _gather_out(res.results, b), res


def build_null_nc(nch=NCH):
    """Same I/O signature as build_nc but near-zero work — measures the
    per-call dispatch overhead so it can be subtracted."""
    nc = bacc.Bacc()
    npad = nch * CH
    nsup = -(-nch // 4)
    fin_d = nc.declare_dram_parameter("fin", [FIN_ROWS, npad], BF16, isOutput=False)
    nc.declare_dram_parameter("wsb", [128, WSB_COLS], BF16, isOutput=False)
    outAB_d = nc.declare_dram_parameter("outAB", [128, 2 * npad], BF16,
                                        isOutput=True)
    nc.declare_dram_parameter("outC", [128, nsup * CH], BF16, isOutput=True)
    with TileContext(nc) as tc:
        with tc.tile_pool(name="t", bufs=1) as pool:
            tl = pool.tile([128, CH], BF16)
            nc.sync.dma_start(out=tl[:], in_=fin_d[0:128, 0:CH])
            nc.sync.dma_start(out=outAB_d[:, 0:CH], in_=tl[:])
    nc.finalize()
    return nc


def _timed_fn(nc, n_loop):
    """Build jitted 8-core executor that runs the NEFF n_loop times per call."""
    import jax
    from concourse import bass2jax as B

    B.install_neuronx_cc_hook()
    partition_name = nc.partition_id_tensor.name if nc.partition_id_tensor else None
    in_names, out_names, out_avals, zero_outs = [], [], [], []
    import concourse.mybir as mb
    for alloc in nc.m.functions[0].allocations:
        if not isinstance(alloc, mb.MemoryLocationSet):
            continue
        name = alloc.memorylocations[0].name
        if alloc.kind == "ExternalInput":
            if name != partition_name:
                in_names.append(name)
        elif alloc.kind == "ExternalOutput":
            shape = tuple(alloc.tensor_shape)
            dtype = mb.dt.np(alloc.dtype)
            out_avals.append(jax.core.ShapedArray(shape, dtype))
            out_names.append(name)
            zero_outs.append(np.zeros(shape, dtype))
    n_params = len(in_names)
    in_names = in_names + out_names
    if partition_name is not None:
        in_names.append(partition_name)

    def _body(*args):
        args = list(args)
        ins = args[:n_params]
        outs = args[n_params:n_params + len(out_names)]
        part = [B.partition_id_tensor()] if partition_name is not None else []
        # Chain n_loop executions through the output buffers: each exec's
        # outputs feed the next exec's out-buffer operands, serializing the
        # NEFF runs so device time is measured n_loop times per jit call.
        for _ in range(n_loop):
            outs = list(B._bass_exec_p.bind(
                *(ins + outs + part),
                out_avals=tuple(out_avals),
                in_names=tuple(in_names),
                out_names=tuple(out_names),
                lowering_input_output_aliases=(),
                sim_require_finite=True,
                sim_require_nnan=True,
                nc=nc,
            ))
        return tuple(outs)

    mesh = B.Mesh(np.asarray(jax.devices()[:N_CORES]), ("core",))
    spec = B.PartitionSpec("core")
    fn = jax.jit(
        B.shard_map(_body, mesh=mesh,
                    in_specs=(spec,) * (n_params + len(out_names)),
                    out_specs=(spec,) * len(out_names), check_rep=False),
        keep_unused=True,
    )
    return fn, mesh, n_params, in_names, zero_outs, out_names


def kernel_timed(atom_agg, res_emb, w, b, backbone_idx, ca_res_idx, pairs=12,
                 r_lo=8, r_hi=40):
    """Returns (out, per_exec_seconds, info). Slope timing: two NEFFs that
    repeat the kernel body r_lo/r_hi times on-device, timed in interleaved
    pairs; per-exec = (median(hi) - median(lo)) / (r_hi - r_lo). Robust to the
    axon tunnel's drifting and bimodal per-call overhead, which cancels in the
    difference."""
    import time

    import jax

    in_maps = _make_in_maps(atom_agg, res_emb, w, b, backbone_idx, ca_res_idx)

    def prep(nc):
        fn, mesh, n_params, in_names, zero_outs, out_names = _timed_fn(nc, 1)
        spec = jax.sharding.NamedSharding(mesh, jax.sharding.PartitionSpec("core"))
        per_core = [[np.asarray(m[n]) for n in in_names[:n_params]] for m in in_maps]
        concat = [np.concatenate([per_core[c][i] for c in range(N_CORES)], 0)
                  for i in range(n_params)]
        concat += [np.zeros((N_CORES * z.shape[0], *z.shape[1:]), z.dtype)
                   for z in zero_outs]
        din = [jax.device_put(x, spec) for x in concat]
        outs = fn(*din)
        jax.block_until_ready(outs)  # compile + warm
        return fn, din, outs, out_names

    fn_lo, din_lo, outs, out_names = prep(build_nc(NCH, reps=r_lo))
    fn_hi, din_hi, _, _ = prep(build_nc(NCH, reps=r_hi))
    los, his = [], []
    for _ in range(pairs):
        t0 = time.perf_counter()
        jax.block_until_ready(fn_lo(*din_lo))
        t1 = time.perf_counter()
        jax.block_until_ready(fn_hi(*din_hi))
        t2 = time.perf_counter()
        los.append(t1 - t0)
        his.append(t2 - t1)

    results = []
    for c in range(N_CORES):
        r = {}
        for i, name in enumerate(out_names):
            full = np.asarray(outs[i])
            per = full.shape[0] // N_CORES
            r[name] = full[c * per:(c + 1) * per]
        results.append(r)
    out_np = _gather_out(results, b)
    med_lo = sorted(los)[len(los) // 2]
    med_hi = sorted(his)[len(his) // 2]
    per_exec = (med_hi - med_lo) / (r_hi - r_lo)
    info = {"r": (r_lo, r_hi),
            "lo_ms": [round(t * 1e3, 2) for t in sorted(los)],
            "hi_ms": [round(t * 1e3, 2) for t in sorted(his)]}
    return out_np, per_exec, info


BUILDERS = {
    "v2_full": lambda: build_nc(NCH),
    "v2_n8": lambda: build_nc(8),
    "null": lambda: build_null_nc(NCH),
}


# revision 4
# speedup vs baseline: 1.0476x; 1.0476x over previous
"""Trainium2 Bass kernel: Atom2Residue (gnn_message_passing).

Math: out[n,c,o] = sum_i fuse[n,c,i] * w[l(c),o,i]  (+ b[o] at c==0)
where fuse[n,c,:] = concat(CA_atom[n,c,:16], res_emb[n,c,:32]), l(c)=floor(sqrt(c)).

Strategy (8 cores, data parallel over 18750 residues/core, no collectives):
  - Host gathers the CA atoms, fuses with res_emb, and pre-packs the per-core
    input TRANSPOSED (channels-on-partitions) in bf16: fin[432, 18944] as 4
    coefficient-pair groups of 96 rows + the c8 group of 48 rows. The device
    does zero on-chip transposes.
  - Per 2048-residue super-chunk: 2 input DMAs, then per 512-residue chunk 5
    block-diagonal matmuls (2 coefs each, K=96/48, bf16, start&stop, no PSUM
    accumulation), PSUM f32 -> SBUF bf16 cast copies split across scalar and
    vector engines, 2 output DMAs per super. Outputs stored transposed; host
    unscrambles, upcasts to f32, adds the l=0 bias.
  - ~41 DMAs/core total (HWDGE descriptor issue is ~625 ns each, serialized
    per ring); input DMAs ride the SP HWDGE ring, output stores the ACT ring
    so stores never head-of-line-block input prefetch.
  - The c8 input group sits on SBUF partitions 64:112 (not 0:48) to balance
    the per-AXI-port byte load: ports are hardwired to partition groups, and
    the main input only covers partitions 0:95.
  - Device HBM traffic: 27.3 MB/core bf16 -> 76 us roofline at 358 GB/s;
    measured ~73-77 us steady-state (vs 1262 us f32 natural-layout baseline).
"""

import os
import sys

for _p in ("/opt/trn_rl_repo",):
    if os.path.isdir(_p) and _p not in sys.path:
        sys.path.insert(0, _p)

import numpy as np

from concourse import bacc, bass, mybir  # noqa: F401
from concourse.bass_utils import run_bass_kernel_spmd
from concourse.tile import TileContext

F32 = mybir.dt.float32
BF16 = mybir.dt.bfloat16
NP_BF16 = mybir.dt.np(mybir.dt.bfloat16)

NUM_COEF, ATOM_C, NODE_C = 9, 16, 32
COEF_C = ATOM_C + NODE_C            # 48 fused in-channels per coefficient
L_OF_COEF = np.floor(np.sqrt(np.arange(NUM_COEF))).astype(np.int64)

N_CORES = 8
R_TOTAL = 150_000
RS = R_TOTAL // N_CORES             # 18750 residues per core
CH = 512                            # residues per chunk (PSUM bank width f32)
NCH = -(-RS // CH)                  # 37 chunks
NPAD = NCH * CH                     # 18944 padded residues per core
FIN_ROWS = NUM_COEF * COEF_C        # 432 input rows (4 groups of 96 + 48)
NSLAB = -(-NCH // 4)                # 10 c8-output slabs of 4 chunks each
WSB_COLS = 4 * 64 + 32              # 288 stationary-weight columns


def build_wsb(w):
    """[128, 288] bf16: 4 groups of block-diag(w[l(2g)].T, w[l(2g+1)].T) 64
    cols each at partition base 0, then w[2].T for coef 8 at col 256 on
    partitions 64:112 (c8 rides the upper partitions to balance the SBUF AXI
    port load of its DMA against the 96-partition main input)."""
    w = np.asarray(w, np.float32)
    wsb = np.zeros((128, WSB_COLS), np.float32)
    for g in range(4):
        for half in range(2):
            c = 2 * g + half
            blk = w[L_OF_COEF[c]].T        # [48 in, 32 out]
            wsb[48 * half:48 * half + 48, 64 * g + 32 * half:64 * g + 32 * half + 32] = blk
    wsb[64:112, 256:288] = w[2].T
    return wsb.astype(NP_BF16)


SUP = 4 * CH                 # 2048-residue super-chunk = one c8 output slab


def build_nc(nch=NCH, reps=1):
    nc = bacc.Bacc()
    npad = nch * CH
    nsup = -(-nch // 4)
    # super widths: 4 chunks each, last one may be partial
    sups = []
    left = nch
    while left > 0:
        w = min(4, left)
        sups.append(w * CH)
        left -= w
    fin_d = nc.declare_dram_parameter("fin", [FIN_ROWS, npad], BF16, isOutput=False)
    wsb_d = nc.declare_dram_parameter("wsb", [128, WSB_COLS], BF16, isOutput=False)
    outAB_d = nc.declare_dram_parameter("outAB", [128, 2 * npad], BF16,
                                        isOutput=True)
    outC_d = nc.declare_dram_parameter("outC", [128, nsup * CH], BF16,
                                       isOutput=True)

    with TileContext(nc) as tc:
        with (
            tc.tile_pool(name="const", bufs=1) as cpool,
            tc.tile_pool(name="fin", bufs=3) as fin_pool,
            tc.tile_pool(name="f4", bufs=3) as f4_pool,
            tc.tile_pool(name="oAB", bufs=3) as oAB_pool,
            tc.tile_pool(name="oC", bufs=2) as oC_pool,
            tc.tile_pool(name="pA", bufs=2, space="PSUM") as pA_pool,
            tc.tile_pool(name="pB", bufs=2, space="PSUM") as pB_pool,
            tc.tile_pool(name="pC", bufs=2, space="PSUM") as pC_pool,
        ):
            wsb_sb = cpool.tile([128, WSB_COLS], BF16)
            nc.sync.dma_start(out=wsb_sb[:], in_=wsb_d[:])

            for s, w in [(s, w) for _ in range(reps) for s, w in enumerate(sups)]:
                c0 = SUP * s                 # residue-column base of the super
                nj = w // CH                 # chunks in this super (4 or less)
                big = fin_pool.tile([96, 4 * w], BF16, tag="big")
                nc.sync.dma_start(
                    out=big[:].rearrange("p (g n) -> p g n", g=4),
                    in_=fin_d[0:384, c0:c0 + w].rearrange("(g p) n -> p g n", g=4),
                )
                f4 = f4_pool.tile([112, w], BF16, tag="f4")
                nc.sync.dma_start(out=f4[64:112, :], in_=fin_d[384:432, c0:c0 + w])

                oAB = oAB_pool.tile([128, 2 * w], BF16, tag="oAB")
                pC = pC_pool.tile([128, CH], F32, tag="pC")
                for j in range(nj):
                    pA = pA_pool.tile([128, CH], F32, tag="pA")
                    pB = pB_pool.tile([128, CH], F32, tag="pB")
                    for g in range(4):
                        ob = 64 * (g % 2)
                        psum = pA if g < 2 else pB
                        nc.tensor.matmul(
                            psum[ob:ob + 64, :],
                            wsb_sb[0:96, 64 * g:64 * g + 64],
                            big[0:96, w * g + CH * j:w * g + CH * (j + 1)],
                            start=True, stop=True,
                            skip_group_check=True,
                            tile_position=(0, ob),
                        )
                    nc.tensor.matmul(
                        pC[32 * j:32 * j + 32, :],
                        wsb_sb[64:112, 256:288],
                        f4[64:112, CH * j:CH * (j + 1)],
                        start=True, stop=True,
                        skip_group_check=True,
                        tile_position=(64, 32 * j),
                    )
                    nc.scalar.copy(out=oAB[:, 2 * CH * j:2 * CH * j + CH],
                                   in_=pA[:])
                    nc.vector.tensor_copy(
                        oAB[:, 2 * CH * j + CH:2 * CH * (j + 1)], pB[:])

                nc.scalar.dma_start(out=outAB_d[:, 2 * c0:2 * (c0 + w)],
                                    in_=oAB[:])
                rows = 32 * nj
                oC = oC_pool.tile([128, CH], BF16, tag="oC")
                if s % 2 == 0:
                    nc.scalar.copy(out=oC[0:rows, :], in_=pC[0:rows, :])
                else:
                    nc.vector.tensor_copy(oC[0:rows, :], pC[0:rows, :])
                nc.scalar.dma_start(out=outC_d[0:rows, CH * s:CH * (s + 1)],
                                    in_=oC[0:rows, :])
    nc.finalize()
    return nc


_NC_CACHE = {}


def _get_nc(nch=NCH):
    if nch not in _NC_CACHE:
        _NC_CACHE[nch] = build_nc(nch)
    return _NC_CACHE[nch]


def _make_in_maps(atom_agg, res_emb, w, b, backbone_idx, ca_res_idx):
    atom_agg = np.ascontiguousarray(np.asarray(atom_agg, dtype=np.float32))
    res_emb = np.ascontiguousarray(np.asarray(res_emb, dtype=np.float32))
    backbone_idx = np.asarray(backbone_idx)
    ca_res_idx = np.asarray(ca_res_idx)
    num_res = res_emb.shape[0]
    assert num_res == R_TOTAL, f"kernel compiled for {R_TOTAL} residues"

    wsb = build_wsb(w)
    A2 = atom_agg.reshape(atom_agg.shape[0], NUM_COEF * ATOM_C)
    ca_atom = backbone_idx.reshape(-1, 4)[:, 1]
    cont = np.zeros((num_res, NUM_COEF * ATOM_C), np.float32)
    cont[ca_res_idx] = A2[ca_atom]

    # bf16 cast in natural layout (contiguous), then one transpose copy into
    # channels-on-partitions [core, 9, 48, NPAD] -> [core, 432, NPAD]
    tmp = np.zeros((N_CORES, NPAD, NUM_COEF, COEF_C), NP_BF16)
    tmp[:, :RS, :, 0:ATOM_C] = cont.reshape(N_CORES, RS, NUM_COEF, ATOM_C)
    tmp[:, :RS, :, ATOM_C:] = res_emb.reshape(N_CORES, RS, NUM_COEF, NODE_C)
    fin_all = np.ascontiguousarray(tmp.transpose(0, 2, 3, 1)).reshape(
        N_CORES, FIN_ROWS, NPAD)

    return [{"fin": fin_all[c], "wsb": wsb} for c in range(N_CORES)]


def _gather_out(results, b):
    b = np.asarray(b, np.float32)
    out = np.empty((N_CORES, RS, NUM_COEF, NODE_C), np.float32)
    for c in range(N_CORES):
        r = results[c]
        # outAB: row 32cc+o, col 1024t + 512h + n ; coef = 4h + cc
        ab = r["outAB"].reshape(4, NODE_C, NCH, 2, CH).transpose(2, 4, 3, 0, 1)
        ab = ab.reshape(NPAD, 2, 4, NODE_C)[:RS]      # [n, h, cc, o]
        out[c, :, 0:4] = np.asarray(ab[:, 0], np.float32)
        out[c, :, 4:8] = np.asarray(ab[:, 1], np.float32)
        # outC: [4(j), 32, NSLAB, 512] ; chunk k=4s+j, n=512k+i
        cC = r["outC"].reshape(4, NODE_C, NSLAB, CH).transpose(2, 0, 3, 1)
        out[c, :, 8] = np.asarray(cC.reshape(NSLAB * 4 * CH, NODE_C)[:RS],
                                  np.float32)
    out = out.reshape(R_TOTAL, NUM_COEF, NODE_C)
    out[:, 0, :] += b
    return out


def _run(in_maps, trace=False, **kw):
    nc = _get_nc()
    return run_bass_kernel_spmd(nc, in_maps, core_ids=list(range(N_CORES)),
                                trace=trace, **kw)


def kernel(atom_agg, res_emb, w, b, backbone_idx, ca_res_idx):
    in_maps = _make_in_maps(atom_agg, res_emb, w, b, backbone_idx, ca_res_idx)
    res = _run(in_maps, trace=False)
    return _gather_out(res.results, b)


def kernel_profiled(atom_agg, res_emb, w, b, backbone_idx, ca_res_idx, **kw):
    """Same as kernel() but requests an NTFF trace; returns (out, BassKernelResults)."""
    in_maps = _make_in_maps(atom_agg, res_emb, w, b, backbone_idx, ca_res_idx)
    res = _run(in_maps, trace=True, **kw)
    return _gather_out(res.results, b), res


def build_null_nc(nch=NCH):
    """Same I/O signature as build_nc but near-zero work — measures the
    per-call dispatch overhead so it can be subtracted."""
    nc = bacc.Bacc()
    npad = nch * CH
    nsup = -(-nch // 4)
    fin_d = nc.declare_dram_parameter("fin", [FIN_ROWS, npad], BF16, isOutput=False)
    nc.declare_dram_parameter("wsb", [128, WSB_COLS], BF16, isOutput=False)
    outAB_d = nc.declare_dram_parameter("outAB", [128, 2 * npad], BF16,
                                        isOutput=True)
    nc.declare_dram_parameter("outC", [128, nsup * CH], BF16, isOutput=True)
    with TileContext(nc) as tc:
        with tc.tile_pool(name="t", bufs=1) as pool:
            tl = pool.tile([128, CH], BF16)
            nc.sync.dma_start(out=tl[:], in_=fin_d[0:128, 0:CH])
            nc.sync.dma_start(out=outAB_d[:, 0:CH], in_=tl[:])
    nc.finalize()
    return nc


def _timed_fn(nc, n_loop):
    """Build jitted 8-core executor that runs the NEFF n_loop times per call."""
    import jax
    from concourse import bass2jax as B

    B.install_neuronx_cc_hook()
    partition_name = nc.partition_id_tensor.name if nc.partition_id_tensor else None
    in_names, out_names, out_avals, zero_outs = [], [], [], []
    import concourse.mybir as mb
    for alloc in nc.m.functions[0].allocations:
        if not isinstance(alloc, mb.MemoryLocationSet):
            continue
        name = alloc.memorylocations[0].name
        if alloc.kind == "ExternalInput":
            if name != partition_name:
                in_names.append(name)
        elif alloc.kind == "ExternalOutput":
            shape = tuple(alloc.tensor_shape)
            dtype = mb.dt.np(alloc.dtype)
            out_avals.append(jax.core.ShapedArray(shape, dtype))
            out_names.append(name)
            zero_outs.append(np.zeros(shape, dtype))
    n_params = len(in_names)
    in_names = in_names + out_names
    if partition_name is not None:
        in_names.append(partition_name)

    def _body(*args):
        args = list(args)
        ins = args[:n_params]
        outs = args[n_params:n_params + len(out_names)]
        part = [B.partition_id_tensor()] if partition_name is not None else []
        # Chain n_loop executions through the output buffers: each exec's
        # outputs feed the next exec's out-buffer operands, serializing the
        # NEFF runs so device time is measured n_loop times per jit call.
        for _ in range(n_loop):
            outs = list(B._bass_exec_p.bind(
                *(ins + outs + part),
                out_avals=tuple(out_avals),
                in_names=tuple(in_names),
                out_names=tuple(out_names),
                lowering_input_output_aliases=(),
                sim_require_finite=True,
                sim_require_nnan=True,
                nc=nc,
            ))
        return tuple(outs)

    mesh = B.Mesh(np.asarray(jax.devices()[:N_CORES]), ("core",))
    spec = B.PartitionSpec("core")
    fn = jax.jit(
        B.shard_map(_body, mesh=mesh,
                    in_specs=(spec,) * (n_params + len(out_names)),
                    out_specs=(spec,) * len(out_names), check_rep=False),
        keep_unused=True,
    )
    return fn, mesh, n_params, in_names, zero_outs, out_names


def kernel_timed(atom_agg, res_emb, w, b, backbone_idx, ca_res_idx, pairs=16,
                 r_lo=8, r_hi=72):
    """Returns (out, per_exec_seconds, info). Slope timing: two NEFFs that
    repeat the kernel body r_lo/r_hi times on-device, timed in interleaved
    pairs; per-exec = (median(hi) - median(lo)) / (r_hi - r_lo). Robust to the
    axon tunnel's drifting and bimodal per-call overhead, which cancels in the
    difference."""
    import time

    import jax

    in_maps = _make_in_maps(atom_agg, res_emb, w, b, backbone_idx, ca_res_idx)

    def prep(nc):
        fn, mesh, n_params, in_names, zero_outs, out_names = _timed_fn(nc, 1)
        spec = jax.sharding.NamedSharding(mesh, jax.sharding.PartitionSpec("core"))
        per_core = [[np.asarray(m[n]) for n in in_names[:n_params]] for m in in_maps]
        concat = [np.concatenate([per_core[c][i] for c in range(N_CORES)], 0)
                  for i in range(n_params)]
        concat += [np.zeros((N_CORES * z.shape[0], *z.shape[1:]), z.dtype)
                   for z in zero_outs]
        din = [jax.device_put(x, spec) for x in concat]
        outs = fn(*din)
        jax.block_until_ready(outs)  # compile + warm
        return fn, din, outs, out_names

    fn_lo, din_lo, outs, out_names = prep(build_nc(NCH, reps=r_lo))
    fn_hi, din_hi, _, _ = prep(build_nc(NCH, reps=r_hi))
    los, his = [], []
    for _ in range(pairs):
        t0 = time.perf_counter()
        jax.block_until_ready(fn_lo(*din_lo))
        t1 = time.perf_counter()
        jax.block_until_ready(fn_hi(*din_hi))
        t2 = time.perf_counter()
        los.append(t1 - t0)
        his.append(t2 - t1)

    results = []
    for c in range(N_CORES):
        r = {}
        for i, name in enumerate(out_names):
            full = np.asarray(outs[i])
            per = full.shape[0] // N_CORES
            r[name] = full[c * per:(c + 1) * per]
        results.append(r)
    out_np = _gather_out(results, b)
    med_lo = sorted(los)[len(los) // 2]
    med_hi = sorted(his)[len(his) // 2]
    per_exec = (med_hi - med_lo) / (r_hi - r_lo)
    info = {"r": (r_lo, r_hi),
            "lo_ms": [round(t * 1e3, 2) for t in sorted(los)],
            "hi_ms": [round(t * 1e3, 2) for t in sorted(his)]}
    return out_np, per_exec, info


BUILDERS = {
    "v2_full": lambda: build_nc(NCH),
    "v2_n8": lambda: build_nc(8),
    "null": lambda: build_null_nc(NCH),
}
